# revision 9
# baseline (speedup 1.0000x reference)
"""CoAttention kernel for Trainium2 (8 NeuronCores, data-parallel over batch).

Math (per sample): ta = relu(seq_a @ W + b), tb likewise.  Mean-pooling the
[N, rv_len, M] affinity commutes with the dot product, so each side only
needs a dot with the other side's per-sample mean feature vector — the 52M
element affinity tensor is never materialized.

v2 design (fp16 on-chip, engines balanced against the ~17 us HBM floor):
 - Host casts seq/W to fp16 and pre-transposes to [300, tokens]; HBM traffic
   halves to ~6.2 MB/core.  PSUM accumulates fp32.
 - FC: c-outer over 3 K-chunks (128/128/44) shared across both sides per
   sample; PSUM windows of 512 (bank rule).  ACT evicts relu to fp16 taT and
   its accum_out gives the token-sum (-> per-sample mean) for free.
 - Scores: 10 accumulating matmuls per sample-side with a block-diagonal
   stationary ([128,10] zeros with the other side's mean in column j) land
   scores directly as PSUM [10,128] — one review per lane, so the whole
   masked softmax runs lane-parallel ([10,128] ops, not [1,1280]).
 - Mask: host precomputes (mask-1)*1e9; a single DVE add applies it.
 - exp on ACT emits fp16 e + fp32 row-sum; ACT scale-copy normalizes to
   fp16 weights.  A scalar-ring SBUF->SBUF DMA flattens [10,128] -> [1,1280]
   (separate HWDGE FIFO from the input stream), gpsimd partition_broadcast
   replicates it across partitions (bitcast to u32 to cut its per-element
   cost), and DVE does fp16 tensor_tensor mult + segmented reduce for the
   weighted sum.  Outputs are written transposed; host flips them back.
"""
import sys

sys.path.insert(0, "/opt/trn_rl_repo")

import numpy as np

import concourse.bacc as bacc
import concourse.tile as tile
from concourse import mybir

# Problem shape (hardcoded per contest contract)
BZ, RV, RL, DIN, DH = 32, 10, 128, 300, 128
NCORES = 8
BPC = BZ // NCORES            # samples per core: 4
TPS = RV * RL                 # tokens per sample-side: 1280
RPC = BPC * RV                # reviews per core: 40
NEG = -1e9

f32 = mybir.dt.float32
f16 = mybir.dt.float16
u32 = mybir.dt.uint32
AF = mybir.ActivationFunctionType
AX = mybir.AxisListType

DCH = [(0, 128), (128, 128), (256, 44)]       # K chunks of DIN=300
NW = [(0, 512), (512, 512), (1024, 256)]      # psum windows within 1280

_CACHE = {}


def _build():
    nc = bacc.Bacc("TRN2", target_bir_lowering=False, debug=False)

    c01 = {s: nc.dram_tensor(f"c01_{s}", [128, 2 * TPS * BPC], f16,
                             kind="ExternalInput") for s in "ab"}
    c2 = {s: nc.dram_tensor(f"c2_{s}", [44, TPS * BPC], f16,
                            kind="ExternalInput") for s in "ab"}
    w_d = nc.dram_tensor("w16", [128, 3 * DH], f16, kind="ExternalInput")
    bias_d = nc.dram_tensor("bias", [DH, 1], f32, kind="ExternalInput")
    mneg_d = nc.dram_tensor("mneg", [RV, 2 * BPC * RL], f32,
                            kind="ExternalInput")

    outv = {s: nc.dram_tensor(f"outv_{s}", [DH, RPC], f32,
                              kind="ExternalOutput") for s in "ab"}
    outw = {s: nc.dram_tensor(f"outw_{s}", [RPC, RL], f16,
                              kind="ExternalOutput") for s in "ab"}
    ssum_d = nc.dram_tensor("ssum", [RV, 2 * BPC], f32,
                            kind="ExternalOutput")

    with tile.TileContext(nc) as tc:
        with (
            tc.tile_pool(name="cst", bufs=1) as cst,
            tc.tile_pool(name="seq", bufs=8) as seqp,
            tc.tile_pool(name="sm", bufs=3) as smpool,
            tc.tile_pool(name="wide", bufs=2) as widep,
            tc.tile_pool(name="ps", bufs=1, space="PSUM") as ps,
        ):
            # ---- constants (scalar/ACT HWDGE ring; sync ring carries seq)
            w16 = cst.tile([128, 3 * DH], f16, tag="w16")
            nc.scalar.dma_start(w16[:], w_d[:])
            bias_t = cst.tile([DH, 1], f32, tag="bias")
            nc.scalar.dma_start(bias_t[:], bias_d[:])
            mneg_t = cst.tile([RV, 2 * BPC * RL], f32, tag="mneg")
            nc.scalar.dma_start(mneg_t[:], mneg_d[:])

            ssumall = cst.tile([RV, 2 * BPC], f32, tag="ssumall",
                               name="ssumall")
            taT, acc, aoutT, stat3 = {}, {}, {}, {}
            for s in "ab":
                taT[s] = cst.tile([DH, BPC * TPS], f16, tag=f"taT{s}", name=f"taT_{s}")
                acc[s] = cst.tile([DH, BPC], f32, tag=f"acc{s}", name=f"acc_{s}")
                aoutT[s] = cst.tile([DH, RPC], f32, tag=f"aoutT{s}", name=f"aoutT_{s}")
                stat3[s] = cst.tile([128, RV, 11], f16, tag=f"stat3{s}", name=f"stat3_{s}")
                nc.vector.memset(stat3[s][:], 0.0)

            # ---- input stream (sync HWDGE ring), sample-pipelined order
            c01_t, c2_t = {}, {}
            for s in "ab":
                c2_t[s] = cst.tile([44, TPS * BPC], f16, tag=f"c2{s}", name=f"c2t_{s}")
            for smp in range(BPC):
                for s in "ba":
                    c01_t[(s, smp)] = seqp.tile([128, 2 * TPS], f16,
                                                tag="c01",
                                                name=f"c01_{s}{smp}")
                    nc.sync.dma_start(
                        c01_t[(s, smp)][:],
                        c01[s][:, smp * 2 * TPS:(smp + 1) * 2 * TPS])
                if smp == 0:
                    for s in "ba":
                        nc.sync.dma_start(c2_t[s][:], c2[s][:])

            pfc = {s: ps.tile([128, 1536], f32, tag=f"pfc{s}", name=f"pfc_{s}")
                   for s in "ab"}

            # PE warm-up: ~44 junk matmuls fill the startup dead-zone and
            # un-throttle the HAM clock gate before real FC work arrives.
            wu_in = cst.tile([128, 512], f16, tag="wu_in", name="wu_in")
            nc.vector.memset(wu_in[:], 0.0)
            wu_ps = ps.tile([RV, 512], f32, tag="psc", bufs=2, name="wu_ps")
            for i in range(12):
                nc.tensor.matmul(wu_ps[:, :], wu_in[:, 0:RV], wu_in[:],
                                 start=True, stop=True)

            other = {"a": "b", "b": "a"}

            def emit_fc(smp):
                t0 = smp * TPS
                # c-outer, sides interleaved: one stationary serves 6 windows
                for c, (d0, dw) in enumerate(DCH):
                    lhsT = w16[0:dw, c * DH:(c + 1) * DH]
                    for s in "ba":
                        for n0, nw in NW:
                            if c < 2:
                                mov = c01_t[(s, smp)][0:dw,
                                                      c * TPS + n0:
                                                      c * TPS + n0 + nw]
                            else:
                                mov = c2_t[s][:, t0 + n0:t0 + n0 + nw]
                            nc.tensor.matmul(pfc[s][:, n0:n0 + nw],
                                             lhsT, mov,
                                             start=(c == 0), stop=(c == 2))
                for s in "ba":
                    nc.scalar.activation(
                        taT[s][:, t0:t0 + TPS], pfc[s][:, 0:TPS], AF.Relu,
                        bias=bias_t[:], accum_out=acc[s][:, smp:smp + 1])

            def emit_tail(smp):
                t0 = smp * TPS
                for si, s in enumerate("ab"):
                    # stationary: zeros [128, 10*11]; mean(other) at col 0 of
                    # each 11-block => slice [10j, 10j+10) has it in col j
                    nc.vector.tensor_scalar_mul(
                        stat3[s][:, :, 0],
                        acc[other[s]][:, smp:smp + 1]
                        .to_broadcast([128, RV]), 1.0 / TPS)
                    statf = stat3[s][:].rearrange("p a b -> p (a b)")
                    psc = ps.tile([RV, RL], f32, tag="psc", bufs=2,
                                  name=f"psc_{s}{smp}")
                    for j in range(RV):
                        nc.tensor.matmul(
                            psc[:, :], statf[:, 10 * j:10 * j + 10],
                            taT[s][:, t0 + j * RL:t0 + (j + 1) * RL],
                            start=(j == 0), stop=(j == RV - 1))
                    # masked logits, lane-parallel [10, 128]
                    mcol = (smp * 2 + si) * RL
                    lgs = smpool.tile([RV, RL], f32, tag="lgs",
                                      name=f"lgs_{s}{smp}")
                    nc.vector.tensor_tensor(
                        out=lgs[:], in0=psc[:, :],
                        in1=mneg_t[:, mcol:mcol + RL],
                        op=mybir.AluOpType.add)
                    negmax = smpool.tile([RV, 1], f32, tag="negmax",
                                         name=f"nm_{s}{smp}")
                    nc.vector.reduce_max(out=negmax[:], in_=lgs[:],
                                         axis=AX.X, negate=True)
                    e16 = smpool.tile([RV, RL], f16, tag="e16",
                                      name=f"e16_{s}{smp}")
                    nc.scalar.activation(
                        e16[:], lgs[:], AF.Exp, bias=negmax[:],
                        accum_out=ssumall[:, smp * 2 + si:smp * 2 + si + 1])
                    # unnormalized weights out as fp16; host divides by ssum
                    nc.sync.dma_start(
                        outw[s][smp * RV:(smp + 1) * RV, :], e16[:])
                    # flatten [10,128] -> [1,1280] on the SWDGE ring
                    erow = smpool.tile([1, TPS], f16, tag="erow",
                                       name=f"erow_{s}{smp}")
                    nc.gpsimd.dma_start(erow[:], e16[:])
                    wbc = widep.tile([128, TPS], f16, tag="wbc",
                                     name=f"wbc_{s}{smp}")
                    nc.gpsimd.partition_broadcast(
                        wbc[:].bitcast(u32), erow[:].bitcast(u32))
                    tmp16 = widep.tile([128, TPS], f16, tag="tmp",
                                       name=f"tmp_{s}{smp}")
                    nc.vector.tensor_tensor(
                        out=tmp16[:], in0=taT[s][:, t0:t0 + TPS],
                        in1=wbc[:], op=mybir.AluOpType.mult)
                    tmp3 = tmp16[:].rearrange("p (r l) -> p r l", r=RV)
                    th = widep.tile([128, RV, RL // 2], f16, tag="th",
                                    name=f"th_{s}{smp}")
                    nc.vector.tensor_tensor(
                        out=th[:], in0=tmp3[:, :, 0:RL // 2],
                        in1=tmp3[:, :, RL // 2:RL],
                        op=mybir.AluOpType.add)
                    nc.vector.reduce_sum(
                        out=aoutT[s][:, smp * RV:(smp + 1) * RV],
                        in_=th[:], axis=AX.X)

            # tails right behind their own sample: the tail engine chain
            # (DVE/ACT/gpsimd) starts draining while later samples stream in.
            for smp in range(BPC):
                emit_fc(smp)
                emit_tail(smp)

            for s in "ab":
                nc.scalar.dma_start(outv[s][:], aoutT[s][:])
            nc.scalar.dma_start(ssum_d[:], ssumall[:])

    nc.compile()
    return nc


def build_in_maps(seq_a, seq_b, mask_a, mask_b, W, b):
    seq_a = np.asarray(seq_a, dtype=np.float32)
    seq_b = np.asarray(seq_b, dtype=np.float32)
    mask_a = np.asarray(mask_a, dtype=np.int32)
    mask_b = np.asarray(mask_b, dtype=np.int32)
    W = np.asarray(W, dtype=np.float32)
    b = np.asarray(b, dtype=np.float32)

    # W chunks as [dw, 128] packed into [128, 384]
    w16 = np.zeros((128, 3 * DH), np.float16)
    for c, (d0, dw) in enumerate(DCH):
        w16[0:dw, c * DH:(c + 1) * DH] = W[d0:d0 + dw, :].astype(np.float16)
    bias_np = np.ascontiguousarray(b.reshape(DH, 1))

    in_maps = []
    for core in range(NCORES):
        b0 = core * BPC
        sl = {"w16": w16, "bias": bias_np}
        for name, seq in (("a", seq_a), ("b", seq_b)):
            chunk = seq[b0:b0 + BPC].reshape(BPC * TPS, DIN)
            t16 = np.ascontiguousarray(chunk.T).astype(np.float16)
            c01 = np.empty((128, 2 * TPS * BPC), np.float16)
            for smp in range(BPC):
                o = smp * 2 * TPS
                c01[:, o:o + TPS] = t16[0:128, smp * TPS:(smp + 1) * TPS]
                c01[:, o + TPS:o + 2 * TPS] = \
                    t16[128:256, smp * TPS:(smp + 1) * TPS]
            sl[f"c01_{name}"] = c01
            sl[f"c2_{name}"] = np.ascontiguousarray(t16[256:300, :])
        # additive mask rows: review r on lane r; col block (smp, side)
        mneg = np.empty((RV, 2 * BPC * RL), np.float32)
        for smp in range(BPC):
            for si, mask in enumerate((mask_a, mask_b)):
                m = mask[b0 + smp].astype(np.float32)  # [RV, RL]
                mneg[:, (smp * 2 + si) * RL:(smp * 2 + si + 1) * RL] = \
                    (m - 1.0) * 1e9
        sl["mneg"] = mneg
        in_maps.append(sl)
    return in_maps


def kernel(seq_a, seq_b, mask_a, mask_b, W, b):
    if "nc" not in _CACHE:
        _CACHE["nc"] = _build()
    nc = _CACHE["nc"]
    in_maps = build_in_maps(seq_a, seq_b, mask_a, mask_b, W, b)

    from concourse.bass_utils import run_bass_kernel_spmd
    res = run_bass_kernel_spmd(nc, in_maps, core_ids=list(range(NCORES)))

    return _gather(res.results)


def _gather(results):
    aa, bb, wa, wb = [], [], [], []
    for r in results:
        ssum = r["ssum"].reshape(RV, BPC, 2)          # [r, smp, side]
        sa = ssum[:, :, 0].T.reshape(RPC, 1)          # a-side, review-major
        sb = ssum[:, :, 1].T.reshape(RPC, 1)
        aa.append(r["outv_a"].T / sa)
        bb.append(r["outv_b"].T / sb)
        wa.append(r["outw_a"].astype(np.float32) / sa)
        wb.append(r["outw_b"].astype(np.float32) / sb)
    return (np.concatenate(aa), np.concatenate(bb),
            np.concatenate(wa), np.concatenate(wb))


# revision 10
# speedup vs baseline: 1.0800x; 1.0800x over previous
"""CoAttention kernel for Trainium2 (8 NeuronCores, data-parallel over batch).

Math (per sample): ta = relu(seq_a @ W + b), tb likewise.  Mean-pooling the
[N, rv_len, M] affinity commutes with the dot product, so each side only
needs a dot with the other side's per-sample mean feature vector — the 52M
element affinity tensor is never materialized.

v2 design (fp16 on-chip, engines balanced against the ~17 us HBM floor):
 - Host casts seq/W to fp16 and pre-transposes to [300, tokens]; HBM traffic
   halves to ~6.2 MB/core.  PSUM accumulates fp32.
 - FC: c-outer over 3 K-chunks (128/128/44) shared across both sides per
   sample; PSUM windows of 512 (bank rule).  ACT evicts relu to fp16 taT and
   its accum_out gives the token-sum (-> per-sample mean) for free.
 - Scores: 10 accumulating matmuls per sample-side with a block-diagonal
   stationary ([128,10] zeros with the other side's mean in column j) land
   scores directly as PSUM [10,128] — one review per lane, so the whole
   masked softmax runs lane-parallel ([10,128] ops, not [1,1280]).
 - Mask: host precomputes (mask-1)*1e9; a single DVE add applies it.
 - exp on ACT emits fp16 e + fp32 row-sum; ACT scale-copy normalizes to
   fp16 weights.  A scalar-ring SBUF->SBUF DMA flattens [10,128] -> [1,1280]
   (separate HWDGE FIFO from the input stream), gpsimd partition_broadcast
   replicates it across partitions (bitcast to u32 to cut its per-element
   cost), and DVE does fp16 tensor_tensor mult + segmented reduce for the
   weighted sum.  Outputs are written transposed; host flips them back.
"""
import sys

sys.path.insert(0, "/opt/trn_rl_repo")

import numpy as np

import concourse.bacc as bacc
import concourse.tile as tile
from concourse import mybir

# Problem shape (hardcoded per contest contract)
BZ, RV, RL, DIN, DH = 32, 10, 128, 300, 128
NCORES = 8
BPC = BZ // NCORES            # samples per core: 4
TPS = RV * RL                 # tokens per sample-side: 1280
RPC = BPC * RV                # reviews per core: 40
NEG = -1e9

f32 = mybir.dt.float32
f16 = mybir.dt.float16
u32 = mybir.dt.uint32
AF = mybir.ActivationFunctionType
AX = mybir.AxisListType

DCH = [(0, 128), (128, 128), (256, 44)]       # K chunks of DIN=300
NW = [(0, 512), (512, 512), (1024, 256)]      # psum windows within 1280

_CACHE = {}


def _build():
    nc = bacc.Bacc("TRN2", target_bir_lowering=False, debug=False)

    c01 = {s: nc.dram_tensor(f"c01_{s}", [128, 2 * TPS * BPC], f16,
                             kind="ExternalInput") for s in "ab"}
    c2 = {s: nc.dram_tensor(f"c2_{s}", [44, TPS * BPC], f16,
                            kind="ExternalInput") for s in "ab"}
    w_d = nc.dram_tensor("w16", [128, 3 * DH], f16, kind="ExternalInput")
    bias_d = nc.dram_tensor("bias", [DH, 1], f32, kind="ExternalInput")
    mneg_d = nc.dram_tensor("mneg", [RV, 2 * BPC * RL], f32,
                            kind="ExternalInput")

    outv = {s: nc.dram_tensor(f"outv_{s}", [DH, RPC], f32,
                              kind="ExternalOutput") for s in "ab"}
    outw = {s: nc.dram_tensor(f"outw_{s}", [RPC, RL], f16,
                              kind="ExternalOutput") for s in "ab"}
    ssum_d = nc.dram_tensor("ssum", [RV, 2 * BPC], f32,
                            kind="ExternalOutput")

    with tile.TileContext(nc) as tc:
        with (
            tc.tile_pool(name="cst", bufs=1) as cst,
            tc.tile_pool(name="seq", bufs=8) as seqp,
            tc.tile_pool(name="sm", bufs=3) as smpool,
            tc.tile_pool(name="wide", bufs=2) as widep,
            tc.tile_pool(name="ps", bufs=1, space="PSUM") as ps,
        ):
            # ---- constants (scalar/ACT HWDGE ring; sync ring carries seq)
            w16 = cst.tile([128, 3 * DH], f16, tag="w16")
            nc.scalar.dma_start(w16[:], w_d[:])
            bias_t = cst.tile([DH, 1], f32, tag="bias")
            nc.scalar.dma_start(bias_t[:], bias_d[:])
            mneg_t = cst.tile([RV, 2 * BPC * RL], f32, tag="mneg")
            nc.scalar.dma_start(mneg_t[:], mneg_d[:])

            ssumall = cst.tile([RV, 2 * BPC], f32, tag="ssumall",
                               name="ssumall")
            taT, acc, aoutT, stat3 = {}, {}, {}, {}
            for s in "ab":
                taT[s] = cst.tile([DH, BPC * TPS], f16, tag=f"taT{s}", name=f"taT_{s}")
                acc[s] = cst.tile([DH, BPC], f32, tag=f"acc{s}", name=f"acc_{s}")
                aoutT[s] = cst.tile([DH, RPC], f32, tag=f"aoutT{s}", name=f"aoutT_{s}")
                stat3[s] = cst.tile([128, RV, 11], f16, tag=f"stat3{s}", name=f"stat3_{s}")
                nc.vector.memset(stat3[s][:], 0.0)

            # ---- input stream (sync HWDGE ring), sample-pipelined order
            c01_t, c2_t = {}, {}
            for s in "ab":
                c2_t[s] = cst.tile([44, TPS * BPC], f16, tag=f"c2{s}", name=f"c2t_{s}")
            for smp in range(BPC):
                for s in "ba":
                    c01_t[(s, smp)] = seqp.tile([128, 2 * TPS], f16,
                                                tag="c01",
                                                name=f"c01_{s}{smp}")
                    nc.sync.dma_start(
                        c01_t[(s, smp)][:],
                        c01[s][:, smp * 2 * TPS:(smp + 1) * 2 * TPS])
                if smp == 0:
                    for s in "ba":
                        nc.sync.dma_start(c2_t[s][:], c2[s][:])

            pfc = {s: ps.tile([128, 1536], f32, tag=f"pfc{s}", name=f"pfc_{s}")
                   for s in "ab"}

            # PE warm-up: ~44 junk matmuls fill the startup dead-zone and
            # un-throttle the HAM clock gate before real FC work arrives.
            wu_in = cst.tile([128, 512], f16, tag="wu_in", name="wu_in")
            nc.vector.memset(wu_in[:], 0.0)
            wu_ps = ps.tile([RV, 512], f32, tag="psc", bufs=2, name="wu_ps")
            for i in range(44):
                nc.tensor.matmul(wu_ps[:, :], wu_in[:, 0:RV], wu_in[:],
                                 start=True, stop=True)

            other = {"a": "b", "b": "a"}

            def emit_fc(smp):
                t0 = smp * TPS
                # c-outer, sides interleaved: one stationary serves 6 windows
                for c, (d0, dw) in enumerate(DCH):
                    lhsT = w16[0:dw, c * DH:(c + 1) * DH]
                    for s in "ba":
                        for n0, nw in NW:
                            if c < 2:
                                mov = c01_t[(s, smp)][0:dw,
                                                      c * TPS + n0:
                                                      c * TPS + n0 + nw]
                            else:
                                mov = c2_t[s][:, t0 + n0:t0 + n0 + nw]
                            nc.tensor.matmul(pfc[s][:, n0:n0 + nw],
                                             lhsT, mov,
                                             start=(c == 0), stop=(c == 2))
                for s in "ba":
                    nc.scalar.activation(
                        taT[s][:, t0:t0 + TPS], pfc[s][:, 0:TPS], AF.Relu,
                        bias=bias_t[:], accum_out=acc[s][:, smp:smp + 1])

            def emit_tail(smp):
                t0 = smp * TPS
                for si, s in enumerate("ab"):
                    # stationary: zeros [128, 10*11]; mean(other) at col 0 of
                    # each 11-block => slice [10j, 10j+10) has it in col j
                    nc.vector.tensor_scalar_mul(
                        stat3[s][:, :, 0],
                        acc[other[s]][:, smp:smp + 1]
                        .to_broadcast([128, RV]), 1.0 / TPS)
                    statf = stat3[s][:].rearrange("p a b -> p (a b)")
                    psc = ps.tile([RV, RL], f32, tag="psc", bufs=2,
                                  name=f"psc_{s}{smp}")
                    for j in range(RV):
                        nc.tensor.matmul(
                            psc[:, :], statf[:, 10 * j:10 * j + 10],
                            taT[s][:, t0 + j * RL:t0 + (j + 1) * RL],
                            start=(j == 0), stop=(j == RV - 1))
                    # masked logits, lane-parallel [10, 128]
                    mcol = (smp * 2 + si) * RL
                    lgs = smpool.tile([RV, RL], f32, tag="lgs",
                                      name=f"lgs_{s}{smp}")
                    nc.vector.tensor_tensor(
                        out=lgs[:], in0=psc[:, :],
                        in1=mneg_t[:, mcol:mcol + RL],
                        op=mybir.AluOpType.add)
                    negmax = smpool.tile([RV, 1], f32, tag="negmax",
                                         name=f"nm_{s}{smp}")
                    nc.vector.reduce_max(out=negmax[:], in_=lgs[:],
                                         axis=AX.X, negate=True)
                    e16 = smpool.tile([RV, RL], f16, tag="e16",
                                      name=f"e16_{s}{smp}")
                    nc.scalar.activation(
                        e16[:], lgs[:], AF.Exp, bias=negmax[:],
                        accum_out=ssumall[:, smp * 2 + si:smp * 2 + si + 1])
                    # unnormalized weights out as fp16; host divides by ssum
                    nc.sync.dma_start(
                        outw[s][smp * RV:(smp + 1) * RV, :], e16[:])
                    # flatten [10,128] -> [1,1280] on the SWDGE ring
                    erow = smpool.tile([1, TPS], f16, tag="erow",
                                       name=f"erow_{s}{smp}")
                    nc.gpsimd.dma_start(erow[:], e16[:])
                    wbc = widep.tile([128, TPS], f16, tag="wbc",
                                     name=f"wbc_{s}{smp}")
                    nc.gpsimd.partition_broadcast(
                        wbc[:].bitcast(u32), erow[:].bitcast(u32))
                    tmp16 = widep.tile([128, TPS], f16, tag="tmp",
                                       name=f"tmp_{s}{smp}")
                    nc.vector.tensor_tensor(
                        out=tmp16[:], in0=taT[s][:, t0:t0 + TPS],
                        in1=wbc[:], op=mybir.AluOpType.mult)
                    tmp3 = tmp16[:].rearrange("p (r l) -> p r l", r=RV)
                    th = widep.tile([128, RV, RL // 2], f16, tag="th",
                                    name=f"th_{s}{smp}")
                    nc.vector.tensor_tensor(
                        out=th[:], in0=tmp3[:, :, 0:RL // 2],
                        in1=tmp3[:, :, RL // 2:RL],
                        op=mybir.AluOpType.add)
                    nc.vector.reduce_sum(
                        out=aoutT[s][:, smp * RV:(smp + 1) * RV],
                        in_=th[:], axis=AX.X)

            # FC one sample ahead of the tail: the in-order PE queue always
            # has matmul work while ACT evicts, keeping the HAM clock warm.
            for smp in range(BPC):
                emit_fc(smp)
                if smp >= 1:
                    emit_tail(smp - 1)
            emit_tail(BPC - 1)

            for s in "ab":
                nc.scalar.dma_start(outv[s][:], aoutT[s][:])
            nc.scalar.dma_start(ssum_d[:], ssumall[:])

    nc.compile()
    return nc


def build_in_maps(seq_a, seq_b, mask_a, mask_b, W, b):
    seq_a = np.asarray(seq_a, dtype=np.float32)
    seq_b = np.asarray(seq_b, dtype=np.float32)
    mask_a = np.asarray(mask_a, dtype=np.int32)
    mask_b = np.asarray(mask_b, dtype=np.int32)
    W = np.asarray(W, dtype=np.float32)
    b = np.asarray(b, dtype=np.float32)

    # W chunks as [dw, 128] packed into [128, 384]
    w16 = np.zeros((128, 3 * DH), np.float16)
    for c, (d0, dw) in enumerate(DCH):
        w16[0:dw, c * DH:(c + 1) * DH] = W[d0:d0 + dw, :].astype(np.float16)
    bias_np = np.ascontiguousarray(b.reshape(DH, 1))

    in_maps = []
    for core in range(NCORES):
        b0 = core * BPC
        sl = {"w16": w16, "bias": bias_np}
        for name, seq in (("a", seq_a), ("b", seq_b)):
            chunk = seq[b0:b0 + BPC].reshape(BPC * TPS, DIN)
            t16 = np.ascontiguousarray(chunk.T).astype(np.float16)
            c01 = np.empty((128, 2 * TPS * BPC), np.float16)
            for smp in range(BPC):
                o = smp * 2 * TPS
                c01[:, o:o + TPS] = t16[0:128, smp * TPS:(smp + 1) * TPS]
                c01[:, o + TPS:o + 2 * TPS] = \
                    t16[128:256, smp * TPS:(smp + 1) * TPS]
            sl[f"c01_{name}"] = c01
            sl[f"c2_{name}"] = np.ascontiguousarray(t16[256:300, :])
        # additive mask rows: review r on lane r; col block (smp, side)
        mneg = np.empty((RV, 2 * BPC * RL), np.float32)
        for smp in range(BPC):
            for si, mask in enumerate((mask_a, mask_b)):
                m = mask[b0 + smp].astype(np.float32)  # [RV, RL]
                mneg[:, (smp * 2 + si) * RL:(smp * 2 + si + 1) * RL] = \
                    (m - 1.0) * 1e9
        sl["mneg"] = mneg
        in_maps.append(sl)
    return in_maps


def kernel(seq_a, seq_b, mask_a, mask_b, W, b):
    if "nc" not in _CACHE:
        _CACHE["nc"] = _build()
    nc = _CACHE["nc"]
    in_maps = build_in_maps(seq_a, seq_b, mask_a, mask_b, W, b)

    from concourse.bass_utils import run_bass_kernel_spmd
    res = run_bass_kernel_spmd(nc, in_maps, core_ids=list(range(NCORES)))

    return _gather(res.results)


def _gather(results):
    aa, bb, wa, wb = [], [], [], []
    for r in results:
        ssum = r["ssum"].reshape(RV, BPC, 2)          # [r, smp, side]
        sa = ssum[:, :, 0].T.reshape(RPC, 1)          # a-side, review-major
        sb = ssum[:, :, 1].T.reshape(RPC, 1)
        aa.append(r["outv_a"].T / sa)
        bb.append(r["outv_b"].T / sb)
        wa.append(r["outw_a"].astype(np.float32) / sa)
        wb.append(r["outw_b"].astype(np.float32) / sb)
    return (np.concatenate(aa), np.concatenate(bb),
            np.concatenate(wa), np.concatenate(wb))


# revision 11
# speedup vs baseline: 1.1320x; 1.0482x over previous
"""CoAttention kernel for Trainium2 (8 NeuronCores, data-parallel over batch).

Math (per sample): ta = relu(seq_a @ W + b), tb likewise.  Mean-pooling the
[N, rv_len, M] affinity commutes with the dot product, so each side only
needs a dot with the other side's per-sample mean feature vector — the 52M
element affinity tensor is never materialized.

v2 design (fp16 on-chip, engines balanced against the ~17 us HBM floor):
 - Host casts seq/W to fp16 and pre-transposes to [300, tokens]; HBM traffic
   halves to ~6.2 MB/core.  PSUM accumulates fp32.
 - FC: c-outer over 3 K-chunks (128/128/44) shared across both sides per
   sample; PSUM windows of 512 (bank rule).  ACT evicts relu to fp16 taT and
   its accum_out gives the token-sum (-> per-sample mean) for free.
 - Scores: 10 accumulating matmuls per sample-side with a block-diagonal
   stationary ([128,10] zeros with the other side's mean in column j) land
   scores directly as PSUM [10,128] — one review per lane, so the whole
   masked softmax runs lane-parallel ([10,128] ops, not [1,1280]).
 - Mask: host precomputes (mask-1)*1e9; a single DVE add applies it.
 - exp on ACT emits fp16 e + fp32 row-sum; ACT scale-copy normalizes to
   fp16 weights.  A scalar-ring SBUF->SBUF DMA flattens [10,128] -> [1,1280]
   (separate HWDGE FIFO from the input stream), gpsimd partition_broadcast
   replicates it across partitions (bitcast to u32 to cut its per-element
   cost), and DVE does fp16 tensor_tensor mult + segmented reduce for the
   weighted sum.  Outputs are written transposed; host flips them back.
"""
import sys

sys.path.insert(0, "/opt/trn_rl_repo")

import numpy as np

import concourse.bacc as bacc
import concourse.tile as tile
from concourse import mybir

# Problem shape (hardcoded per contest contract)
BZ, RV, RL, DIN, DH = 32, 10, 128, 300, 128
NCORES = 8
BPC = BZ // NCORES            # samples per core: 4
TPS = RV * RL                 # tokens per sample-side: 1280
RPC = BPC * RV                # reviews per core: 40
NEG = -1e9

f32 = mybir.dt.float32
f16 = mybir.dt.float16
u32 = mybir.dt.uint32
u64 = mybir.dt.uint64
AF = mybir.ActivationFunctionType
AX = mybir.AxisListType

DCH = [(0, 128), (128, 128), (256, 44)]       # K chunks of DIN=300
NW = [(0, 512), (512, 512), (1024, 256)]      # psum windows within 1280

_CACHE = {}


def _build():
    nc = bacc.Bacc("TRN2", target_bir_lowering=False, debug=False)

    c01 = {s: nc.dram_tensor(f"c01_{s}", [128, 2 * TPS * BPC], f16,
                             kind="ExternalInput") for s in "ab"}
    c2 = {s: nc.dram_tensor(f"c2_{s}", [44, TPS * BPC], f16,
                            kind="ExternalInput") for s in "ab"}
    w_d = nc.dram_tensor("w16", [128, 3 * DH], f16, kind="ExternalInput")
    bias_d = nc.dram_tensor("bias", [DH, 1], f32, kind="ExternalInput")
    mneg_d = nc.dram_tensor("mneg", [RV, 2 * BPC * RL], f32,
                            kind="ExternalInput")

    outv = {s: nc.dram_tensor(f"outv_{s}", [DH, RPC], f32,
                              kind="ExternalOutput") for s in "ab"}
    outw = {s: nc.dram_tensor(f"outw_{s}", [RPC, RL], f16,
                              kind="ExternalOutput") for s in "ab"}
    ssum_d = nc.dram_tensor("ssum", [RV, 2 * BPC], f32,
                            kind="ExternalOutput")

    with tile.TileContext(nc) as tc:
        with (
            tc.tile_pool(name="cst", bufs=1) as cst,
            tc.tile_pool(name="seq", bufs=8) as seqp,
            tc.tile_pool(name="sm", bufs=3) as smpool,
            tc.tile_pool(name="ps", bufs=1, space="PSUM") as ps,
        ):
            # ---- constants (scalar/ACT HWDGE ring; sync ring carries seq)
            w16 = cst.tile([128, 3 * DH], f16, tag="w16")
            nc.scalar.dma_start(w16[:], w_d[:])
            bias_t = cst.tile([DH, 1], f32, tag="bias")
            nc.scalar.dma_start(bias_t[:], bias_d[:])
            mneg_t = cst.tile([RV, 2 * BPC * RL], f32, tag="mneg")
            nc.scalar.dma_start(mneg_t[:], mneg_d[:])

            ssumall = cst.tile([RV, 2 * BPC], f32, tag="ssumall",
                               name="ssumall")
            taT, acc, aoutT, stat3 = {}, {}, {}, {}
            for s in "ab":
                taT[s] = cst.tile([DH, BPC * TPS], f16, tag=f"taT{s}", name=f"taT_{s}")
                acc[s] = cst.tile([DH, BPC], f32, tag=f"acc{s}", name=f"acc_{s}")
                aoutT[s] = cst.tile([DH, RPC], f32, tag=f"aoutT{s}", name=f"aoutT_{s}")
                stat3[s] = cst.tile([128, RV, 11], f16, tag=f"stat3{s}", name=f"stat3_{s}")
                nc.vector.memset(stat3[s][:], 0.0)

            # ---- input stream (sync HWDGE ring), sample-pipelined order
            c01_t, c2_t = {}, {}
            for s in "ab":
                c2_t[s] = cst.tile([44, TPS * BPC], f16, tag=f"c2{s}", name=f"c2t_{s}")
            for smp in range(BPC):
                for s in "ba":
                    c01_t[(s, smp)] = seqp.tile([128, 2 * TPS], f16,
                                                tag="c01",
                                                name=f"c01_{s}{smp}")
                    nc.sync.dma_start(
                        c01_t[(s, smp)][:],
                        c01[s][:, smp * 2 * TPS:(smp + 1) * 2 * TPS])
                if smp == 0:
                    for s in "ba":
                        nc.sync.dma_start(c2_t[s][:], c2[s][:])

            pfc = {s: ps.tile([128, 1536], f32, tag=f"pfc{s}", name=f"pfc_{s}")
                   for s in "ab"}

            # PE warm-up: ~44 junk matmuls fill the startup dead-zone and
            # un-throttle the HAM clock gate before real FC work arrives.
            wu_in = cst.tile([128, 512], f16, tag="wu_in", name="wu_in")
            nc.vector.memset(wu_in[:], 0.0)
            wu_ps = ps.tile([RV, 512], f32, tag="psc", bufs=2, name="wu_ps")
            for i in range(6):
                nc.tensor.matmul(wu_ps[:, :], wu_in[:, 0:RV], wu_in[:],
                                 start=True, stop=True)

            other = {"a": "b", "b": "a"}

            def emit_fc(smp):
                t0 = smp * TPS
                # c-outer, sides interleaved: one stationary serves 6 windows
                for c, (d0, dw) in enumerate(DCH):
                    lhsT = w16[0:dw, c * DH:(c + 1) * DH]
                    for s in "ba":
                        for n0, nw in NW:
                            if c < 2:
                                mov = c01_t[(s, smp)][0:dw,
                                                      c * TPS + n0:
                                                      c * TPS + n0 + nw]
                            else:
                                mov = c2_t[s][:, t0 + n0:t0 + n0 + nw]
                            nc.tensor.matmul(pfc[s][:, n0:n0 + nw],
                                             lhsT, mov,
                                             start=(c == 0), stop=(c == 2))
                for s in "ba":
                    nc.scalar.activation(
                        taT[s][:, t0:t0 + TPS], pfc[s][:, 0:TPS], AF.Relu,
                        bias=bias_t[:], accum_out=acc[s][:, smp:smp + 1])

            def emit_tail(smp):
                t0 = smp * TPS
                for si, s in enumerate("ab"):
                    # stationary: zeros [128, 10*11]; mean(other) at col 0 of
                    # each 11-block => slice [10j, 10j+10) has it in col j
                    nc.vector.tensor_scalar_mul(
                        stat3[s][:, :, 0],
                        acc[other[s]][:, smp:smp + 1]
                        .to_broadcast([128, RV]), 1.0 / TPS)
                    statf = stat3[s][:].rearrange("p a b -> p (a b)")
                    psc = ps.tile([RV, RL], f32, tag="psc", bufs=2,
                                  name=f"psc_{s}{smp}")
                    for j in range(RV):
                        nc.tensor.matmul(
                            psc[:, :], statf[:, 10 * j:10 * j + 10],
                            taT[s][:, t0 + j * RL:t0 + (j + 1) * RL],
                            start=(j == 0), stop=(j == RV - 1))
                    # masked logits, lane-parallel [10, 128]
                    mcol = (smp * 2 + si) * RL
                    lgs = smpool.tile([RV, RL], f32, tag="lgs",
                                      name=f"lgs_{s}{smp}")
                    nc.vector.tensor_tensor(
                        out=lgs[:], in0=psc[:, :],
                        in1=mneg_t[:, mcol:mcol + RL],
                        op=mybir.AluOpType.add)
                    negmax = smpool.tile([RV, 1], f32, tag="negmax",
                                         name=f"nm_{s}{smp}")
                    nc.vector.reduce_max(out=negmax[:], in_=lgs[:],
                                         axis=AX.X, negate=True)
                    e16 = smpool.tile([RV, RL], f16, tag="e16",
                                      name=f"e16_{s}{smp}")
                    nc.scalar.activation(
                        e16[:], lgs[:], AF.Exp, bias=negmax[:],
                        accum_out=ssumall[:, smp * 2 + si:smp * 2 + si + 1])
                    # unnormalized weights out as fp16; host divides by ssum
                    nc.sync.dma_start(
                        outw[s][smp * RV:(smp + 1) * RV, :], e16[:])
                    # flatten [10,128] -> [1,1280] on the scalar HWDGE ring
                    erow = smpool.tile([1, TPS], f16, tag="erow",
                                       name=f"erow_{s}{smp}")
                    nc.scalar.dma_start(erow[:], e16[:])
                    wbc = smpool.tile([128, TPS], f16, tag="wbc", bufs=2,
                                     name=f"wbc_{s}{smp}")
                    nc.gpsimd.partition_broadcast(
                        wbc[:].bitcast(u64), erow[:].bitcast(u64))
                    tmp16 = smpool.tile([128, TPS], f16, tag="tmp", bufs=2,
                                       name=f"tmp_{s}{smp}")
                    nc.vector.tensor_tensor(
                        out=tmp16[:], in0=taT[s][:, t0:t0 + TPS],
                        in1=wbc[:], op=mybir.AluOpType.mult)
                    tmp3 = tmp16[:].rearrange("p (r l) -> p r l", r=RV)
                    th = smpool.tile([128, RV, RL // 2], f16, tag="th", bufs=2,
                                    name=f"th_{s}{smp}")
                    nc.vector.tensor_tensor(
                        out=th[:], in0=tmp3[:, :, 0:RL // 2],
                        in1=tmp3[:, :, RL // 2:RL],
                        op=mybir.AluOpType.add)
                    nc.vector.reduce_sum(
                        out=aoutT[s][:, smp * RV:(smp + 1) * RV],
                        in_=th[:], axis=AX.X)

            # FC one sample ahead of the tail: the in-order PE queue always
            # has matmul work while ACT evicts, keeping the HAM clock warm.
            for smp in range(BPC):
                emit_fc(smp)
                if smp >= 1:
                    emit_tail(smp - 1)
            emit_tail(BPC - 1)

            for s in "ab":
                nc.scalar.dma_start(outv[s][:], aoutT[s][:])
            nc.scalar.dma_start(ssum_d[:], ssumall[:])

    nc.compile()
    return nc


def build_in_maps(seq_a, seq_b, mask_a, mask_b, W, b):
    seq_a = np.asarray(seq_a, dtype=np.float32)
    seq_b = np.asarray(seq_b, dtype=np.float32)
    mask_a = np.asarray(mask_a, dtype=np.int32)
    mask_b = np.asarray(mask_b, dtype=np.int32)
    W = np.asarray(W, dtype=np.float32)
    b = np.asarray(b, dtype=np.float32)

    # W chunks as [dw, 128] packed into [128, 384]
    w16 = np.zeros((128, 3 * DH), np.float16)
    for c, (d0, dw) in enumerate(DCH):
        w16[0:dw, c * DH:(c + 1) * DH] = W[d0:d0 + dw, :].astype(np.float16)
    bias_np = np.ascontiguousarray(b.reshape(DH, 1))

    in_maps = []
    for core in range(NCORES):
        b0 = core * BPC
        sl = {"w16": w16, "bias": bias_np}
        for name, seq in (("a", seq_a), ("b", seq_b)):
            chunk = seq[b0:b0 + BPC].reshape(BPC * TPS, DIN)
            t16 = np.ascontiguousarray(chunk.T).astype(np.float16)
            c01 = np.empty((128, 2 * TPS * BPC), np.float16)
            for smp in range(BPC):
                o = smp * 2 * TPS
                c01[:, o:o + TPS] = t16[0:128, smp * TPS:(smp + 1) * TPS]
                c01[:, o + TPS:o + 2 * TPS] = \
                    t16[128:256, smp * TPS:(smp + 1) * TPS]
            sl[f"c01_{name}"] = c01
            sl[f"c2_{name}"] = np.ascontiguousarray(t16[256:300, :])
        # additive mask rows: review r on lane r; col block (smp, side)
        mneg = np.empty((RV, 2 * BPC * RL), np.float32)
        for smp in range(BPC):
            for si, mask in enumerate((mask_a, mask_b)):
                m = mask[b0 + smp].astype(np.float32)  # [RV, RL]
                mneg[:, (smp * 2 + si) * RL:(smp * 2 + si + 1) * RL] = \
                    (m - 1.0) * 1e9
        sl["mneg"] = mneg
        in_maps.append(sl)
    return in_maps


def kernel(seq_a, seq_b, mask_a, mask_b, W, b):
    if "nc" not in _CACHE:
        _CACHE["nc"] = _build()
    nc = _CACHE["nc"]
    in_maps = build_in_maps(seq_a, seq_b, mask_a, mask_b, W, b)

    from concourse.bass_utils import run_bass_kernel_spmd
    res = run_bass_kernel_spmd(nc, in_maps, core_ids=list(range(NCORES)))

    return _gather(res.results)


def _gather(results):
    aa, bb, wa, wb = [], [], [], []
    for r in results:
        ssum = r["ssum"].reshape(RV, BPC, 2)          # [r, smp, side]
        sa = ssum[:, :, 0].T.reshape(RPC, 1)          # a-side, review-major
        sb = ssum[:, :, 1].T.reshape(RPC, 1)
        aa.append(r["outv_a"].T / sa)
        bb.append(r["outv_b"].T / sb)
        wa.append(r["outw_a"].astype(np.float32) / sa)
        wb.append(r["outw_b"].astype(np.float32) / sb)
    return (np.concatenate(aa), np.concatenate(bb),
            np.concatenate(wa), np.concatenate(wb))


# revision 12
# speedup vs baseline: 1.1643x; 1.0285x over previous
"""CoAttention kernel for Trainium2 (8 NeuronCores, data-parallel over batch).

Math (per sample): ta = relu(seq_a @ W + b), tb likewise.  Mean-pooling the
[N, rv_len, M] affinity commutes with the dot product, so each side only
needs a dot with the other side's per-sample mean feature vector — the 52M
element affinity tensor is never materialized.

v2 design (fp16 on-chip, engines balanced against the ~17 us HBM floor):
 - Host casts seq/W to fp16 and pre-transposes to [300, tokens]; HBM traffic
   halves to ~6.2 MB/core.  PSUM accumulates fp32.
 - FC: c-outer over 3 K-chunks (128/128/44) shared across both sides per
   sample; PSUM windows of 512 (bank rule).  ACT evicts relu to fp16 taT and
   its accum_out gives the token-sum (-> per-sample mean) for free.
 - Scores: 10 accumulating matmuls per sample-side with a block-diagonal
   stationary ([128,10] zeros with the other side's mean in column j) land
   scores directly as PSUM [10,128] — one review per lane, so the whole
   masked softmax runs lane-parallel ([10,128] ops, not [1,1280]).
 - Mask: host precomputes (mask-1)*1e9; a single DVE add applies it.
 - exp on ACT emits fp16 e + fp32 row-sum; ACT scale-copy normalizes to
   fp16 weights.  A scalar-ring SBUF->SBUF DMA flattens [10,128] -> [1,1280]
   (separate HWDGE FIFO from the input stream), gpsimd partition_broadcast
   replicates it across partitions (bitcast to u32 to cut its per-element
   cost), and DVE does fp16 tensor_tensor mult + segmented reduce for the
   weighted sum.  Outputs are written transposed; host flips them back.
"""
import sys

sys.path.insert(0, "/opt/trn_rl_repo")

import numpy as np

import concourse.bacc as bacc
import concourse.tile as tile
from concourse import mybir

# Problem shape (hardcoded per contest contract)
BZ, RV, RL, DIN, DH = 32, 10, 128, 300, 128
NCORES = 8
BPC = BZ // NCORES            # samples per core: 4
TPS = RV * RL                 # tokens per sample-side: 1280
RPC = BPC * RV                # reviews per core: 40
NEG = -1e9

f32 = mybir.dt.float32
f16 = mybir.dt.float16
u32 = mybir.dt.uint32
u64 = mybir.dt.uint64
AF = mybir.ActivationFunctionType
AX = mybir.AxisListType

DCH = [(0, 128), (128, 128), (256, 44)]       # K chunks of DIN=300
NW = [(0, 512), (512, 512), (1024, 256)]      # psum windows within 1280

_CACHE = {}


def _build():
    nc = bacc.Bacc("TRN2", target_bir_lowering=False, debug=False)

    c01 = {s: nc.dram_tensor(f"c01_{s}", [128, 2 * TPS * BPC], f16,
                             kind="ExternalInput") for s in "ab"}
    c2 = {s: nc.dram_tensor(f"c2_{s}", [44, TPS * BPC], f16,
                            kind="ExternalInput") for s in "ab"}
    w_d = nc.dram_tensor("w16", [128, 3 * DH], f16, kind="ExternalInput")
    bias_d = nc.dram_tensor("bias", [DH, 1], f32, kind="ExternalInput")
    mneg_d = nc.dram_tensor("mneg", [RV, 2 * BPC * RL], f32,
                            kind="ExternalInput")

    outv = {s: nc.dram_tensor(f"outv_{s}", [DH, RPC], f32,
                              kind="ExternalOutput") for s in "ab"}
    outw = {s: nc.dram_tensor(f"outw_{s}", [RPC, RL], f16,
                              kind="ExternalOutput") for s in "ab"}
    ssum_d = nc.dram_tensor("ssum", [RV, 2 * BPC], f32,
                            kind="ExternalOutput")

    with tile.TileContext(nc) as tc:
        with (
            tc.tile_pool(name="cst", bufs=1) as cst,
            tc.tile_pool(name="seq", bufs=8) as seqp,
            tc.tile_pool(name="sm", bufs=3) as smpool,
            tc.tile_pool(name="ps", bufs=1, space="PSUM") as ps,
        ):
            # ---- constants (scalar/ACT HWDGE ring; sync ring carries seq)
            w16 = cst.tile([128, 3 * DH], f16, tag="w16")
            nc.scalar.dma_start(w16[:], w_d[:])
            bias_t = cst.tile([DH, 1], f32, tag="bias")
            nc.scalar.dma_start(bias_t[:], bias_d[:])
            mneg_t = cst.tile([RV, 2 * BPC * RL], f32, tag="mneg")
            nc.scalar.dma_start(mneg_t[:], mneg_d[:])

            ssumall = cst.tile([RV, 2 * BPC], f32, tag="ssumall",
                               name="ssumall")
            taT, acc, aoutT, stat3 = {}, {}, {}, {}
            for s in "ab":
                taT[s] = cst.tile([DH, BPC * TPS], f16, tag=f"taT{s}", name=f"taT_{s}")
                acc[s] = cst.tile([DH, BPC], f32, tag=f"acc{s}", name=f"acc_{s}")
                aoutT[s] = cst.tile([DH, RPC], f32, tag=f"aoutT{s}", name=f"aoutT_{s}")
                stat3[s] = cst.tile([128, RV, 11], f16, tag=f"stat3{s}", name=f"stat3_{s}")
                nc.vector.memset(stat3[s][:], 0.0)

            # ---- input stream (sync HWDGE ring), sample-pipelined order
            c01_t, c2_t = {}, {}
            for s in "ab":
                c2_t[s] = cst.tile([44, TPS * BPC], f16, tag=f"c2{s}", name=f"c2t_{s}")
            for smp in range(BPC):
                for s in "ba":
                    c01_t[(s, smp)] = seqp.tile([128, 2 * TPS], f16,
                                                tag="c01",
                                                name=f"c01_{s}{smp}")
                    nc.sync.dma_start(
                        c01_t[(s, smp)][:],
                        c01[s][:, smp * 2 * TPS:(smp + 1) * 2 * TPS])
                if smp == 0:
                    for s in "ba":
                        nc.sync.dma_start(c2_t[s][:], c2[s][:])

            pfc = {s: ps.tile([128, 1536], f32, tag=f"pfc{s}", name=f"pfc_{s}")
                   for s in "ab"}

            # PE warm-up: ~44 junk matmuls fill the startup dead-zone and
            # un-throttle the HAM clock gate before real FC work arrives.
            wu_in = cst.tile([128, 512], f16, tag="wu_in", name="wu_in")
            nc.vector.memset(wu_in[:], 0.0)
            wu_ps = ps.tile([RV, 512], f32, tag="psc", bufs=2, name="wu_ps")
            for i in range(12):
                nc.tensor.matmul(wu_ps[:, :], wu_in[:, 0:RV], wu_in[:],
                                 start=True, stop=True)

            other = {"a": "b", "b": "a"}

            def emit_fc(smp):
                t0 = smp * TPS
                # c-outer, sides interleaved: one stationary serves 6 windows
                for c, (d0, dw) in enumerate(DCH):
                    lhsT = w16[0:dw, c * DH:(c + 1) * DH]
                    for s in "ba":
                        for n0, nw in NW:
                            if c < 2:
                                mov = c01_t[(s, smp)][0:dw,
                                                      c * TPS + n0:
                                                      c * TPS + n0 + nw]
                            else:
                                mov = c2_t[s][:, t0 + n0:t0 + n0 + nw]
                            nc.tensor.matmul(pfc[s][:, n0:n0 + nw],
                                             lhsT, mov,
                                             start=(c == 0), stop=(c == 2))
                for s in "ba":
                    nc.scalar.activation(
                        taT[s][:, t0:t0 + TPS], pfc[s][:, 0:TPS], AF.Relu,
                        bias=bias_t[:], accum_out=acc[s][:, smp:smp + 1])

            def emit_tail(smp):
                t0 = smp * TPS
                for si, s in enumerate("ab"):
                    # stationary: zeros [128, 10*11]; mean(other) at col 0 of
                    # each 11-block => slice [10j, 10j+10) has it in col j
                    nc.vector.tensor_scalar_mul(
                        stat3[s][:, :, 0],
                        acc[other[s]][:, smp:smp + 1]
                        .to_broadcast([128, RV]), 1.0 / TPS)
                    statf = stat3[s][:].rearrange("p a b -> p (a b)")
                    psc = ps.tile([RV, RL], f32, tag="psc", bufs=2,
                                  name=f"psc_{s}{smp}")
                    for j in range(RV):
                        nc.tensor.matmul(
                            psc[:, :], statf[:, 10 * j:10 * j + 10],
                            taT[s][:, t0 + j * RL:t0 + (j + 1) * RL],
                            start=(j == 0), stop=(j == RV - 1))
                    # masked logits, lane-parallel [10, 128]
                    mcol = (smp * 2 + si) * RL
                    lgs = smpool.tile([RV, RL], f32, tag="lgs",
                                      name=f"lgs_{s}{smp}")
                    nc.vector.tensor_tensor(
                        out=lgs[:], in0=psc[:, :],
                        in1=mneg_t[:, mcol:mcol + RL],
                        op=mybir.AluOpType.add)
                    negmax = smpool.tile([RV, 1], f32, tag="negmax",
                                         name=f"nm_{s}{smp}")
                    nc.vector.reduce_max(out=negmax[:], in_=lgs[:],
                                         axis=AX.X, negate=True)
                    e16 = smpool.tile([RV, RL], f16, tag="e16",
                                      name=f"e16_{s}{smp}")
                    nc.scalar.activation(
                        e16[:], lgs[:], AF.Exp, bias=negmax[:],
                        accum_out=ssumall[:, smp * 2 + si:smp * 2 + si + 1])
                    # unnormalized weights out as fp16; host divides by ssum
                    nc.sync.dma_start(
                        outw[s][smp * RV:(smp + 1) * RV, :], e16[:])
                    # flatten [10,128] -> [1,1280] on the scalar HWDGE ring
                    erow = smpool.tile([1, TPS], f16, tag="erow",
                                       name=f"erow_{s}{smp}")
                    nc.scalar.dma_start(erow[:], e16[:])
                    wbc = smpool.tile([128, TPS], f16, tag="wbc", bufs=2,
                                     name=f"wbc_{s}{smp}")
                    nc.gpsimd.partition_broadcast(
                        wbc[:].bitcast(u64), erow[:].bitcast(u64))
                    tmp16 = smpool.tile([128, TPS], f16, tag="tmp", bufs=2,
                                       name=f"tmp_{s}{smp}")
                    nc.vector.tensor_tensor(
                        out=tmp16[:], in0=taT[s][:, t0:t0 + TPS],
                        in1=wbc[:], op=mybir.AluOpType.mult)
                    tmp3 = tmp16[:].rearrange("p (r l) -> p r l", r=RV)
                    th = smpool.tile([128, RV, RL // 2], f16, tag="th", bufs=2,
                                    name=f"th_{s}{smp}")
                    nc.vector.tensor_tensor(
                        out=th[:], in0=tmp3[:, :, 0:RL // 2],
                        in1=tmp3[:, :, RL // 2:RL],
                        op=mybir.AluOpType.add)
                    nc.vector.reduce_sum(
                        out=aoutT[s][:, smp * RV:(smp + 1) * RV],
                        in_=th[:], axis=AX.X)

            # FC one sample ahead of the tail: the in-order PE queue always
            # has matmul work while ACT evicts, keeping the HAM clock warm.
            for smp in range(BPC):
                emit_fc(smp)
                if smp >= 1:
                    emit_tail(smp - 1)
            emit_tail(BPC - 1)

            for s in "ab":
                nc.scalar.dma_start(outv[s][:], aoutT[s][:])
            nc.scalar.dma_start(ssum_d[:], ssumall[:])

    nc.compile()
    return nc


def build_in_maps(seq_a, seq_b, mask_a, mask_b, W, b):
    seq_a = np.asarray(seq_a, dtype=np.float32)
    seq_b = np.asarray(seq_b, dtype=np.float32)
    mask_a = np.asarray(mask_a, dtype=np.int32)
    mask_b = np.asarray(mask_b, dtype=np.int32)
    W = np.asarray(W, dtype=np.float32)
    b = np.asarray(b, dtype=np.float32)

    # W chunks as [dw, 128] packed into [128, 384]
    w16 = np.zeros((128, 3 * DH), np.float16)
    for c, (d0, dw) in enumerate(DCH):
        w16[0:dw, c * DH:(c + 1) * DH] = W[d0:d0 + dw, :].astype(np.float16)
    bias_np = np.ascontiguousarray(b.reshape(DH, 1))

    in_maps = []
    for core in range(NCORES):
        b0 = core * BPC
        sl = {"w16": w16, "bias": bias_np}
        for name, seq in (("a", seq_a), ("b", seq_b)):
            chunk = seq[b0:b0 + BPC].reshape(BPC * TPS, DIN)
            t16 = np.ascontiguousarray(chunk.T).astype(np.float16)
            c01 = np.empty((128, 2 * TPS * BPC), np.float16)
            for smp in range(BPC):
                o = smp * 2 * TPS
                c01[:, o:o + TPS] = t16[0:128, smp * TPS:(smp + 1) * TPS]
                c01[:, o + TPS:o + 2 * TPS] = \
                    t16[128:256, smp * TPS:(smp + 1) * TPS]
            sl[f"c01_{name}"] = c01
            sl[f"c2_{name}"] = np.ascontiguousarray(t16[256:300, :])
        # additive mask rows: review r on lane r; col block (smp, side)
        mneg = np.empty((RV, 2 * BPC * RL), np.float32)
        for smp in range(BPC):
            for si, mask in enumerate((mask_a, mask_b)):
                m = mask[b0 + smp].astype(np.float32)  # [RV, RL]
                mneg[:, (smp * 2 + si) * RL:(smp * 2 + si + 1) * RL] = \
                    (m - 1.0) * 1e9
        sl["mneg"] = mneg
        in_maps.append(sl)
    return in_maps


def kernel(seq_a, seq_b, mask_a, mask_b, W, b):
    if "nc" not in _CACHE:
        _CACHE["nc"] = _build()
    nc = _CACHE["nc"]
    in_maps = build_in_maps(seq_a, seq_b, mask_a, mask_b, W, b)

    from concourse.bass_utils import run_bass_kernel_spmd
    res = run_bass_kernel_spmd(nc, in_maps, core_ids=list(range(NCORES)))

    return _gather(res.results)


def _gather(results):
    aa, bb, wa, wb = [], [], [], []
    for r in results:
        ssum = r["ssum"].reshape(RV, BPC, 2)          # [r, smp, side]
        sa = ssum[:, :, 0].T.reshape(RPC, 1)          # a-side, review-major
        sb = ssum[:, :, 1].T.reshape(RPC, 1)
        aa.append(r["outv_a"].T / sa)
        bb.append(r["outv_b"].T / sb)
        wa.append(r["outw_a"].astype(np.float32) / sa)
        wb.append(r["outw_b"].astype(np.float32) / sb)
    return (np.concatenate(aa), np.concatenate(bb),
            np.concatenate(wa), np.concatenate(wb))
